# revision 1
# baseline (speedup 1.0000x reference)
"""Trainium2 Bass kernel for nn_LongTermMemoryMLP.

Per-batch-weight 3-layer MLP:
    h0 = relu(q @ W0^T + b0); h1 = relu(h0 @ W1^T + b1); out = h1 @ W2^T + b2
with q: [B,S,DIN], W0: [B,DH,DIN], W1: [B,DH,DH], W2: [B,DOUT,DH], B=8.

Sharding: data-parallel over batch — one batch sample (and its weight slabs)
per NeuronCore, 8 cores, no cross-core communication.

Device-side strategy: activations are kept feature-major ([feature, seq],
feature on partitions) so every layer is a plain accumulated matmul with the
(pre-transposed) weights as the stationary operand and the activations as the
moving operand — no on-chip transposes. The final layer flips orientation
(stationary = activation tile, moving = W2^T) so the output lands seq-major
and can be DMA'd out contiguously. Inputs are pre-transposed on the host.
Matmuls run as float32r (full fp32 storage, PE rounds internally to ~11-12
mantissa bits, streams at 1 row/cycle for N>=256): ~16x more accurate than
bf16 at ~10% more PE time, fp32 accumulation in PSUM.
"""

import numpy as np

import ml_dtypes

import concourse.bass as bass
import concourse.tile as tile
from concourse import bacc, mybir
from concourse.bass_utils import run_bass_kernel_spmd

B, S, DIN, DH, DOUT = 8, 4096, 512, 1024, 512
SC = 512  # seq chunk processed per pipeline iteration

BF16 = mybir.dt.bfloat16
F32 = mybir.dt.float32
F32R = mybir.dt.float32r


def build_nc():
    nc = bacc.Bacc("TRN2")
    qT = nc.dram_tensor("qT", (DIN, S), F32R, kind="ExternalInput")
    w0t = nc.dram_tensor("w0t", (DIN, DH), F32R, kind="ExternalInput")
    w1t = nc.dram_tensor("w1t", (DH, DH), F32R, kind="ExternalInput")
    w2t = nc.dram_tensor("w2t", (DH, DOUT), F32R, kind="ExternalInput")
    b0 = nc.dram_tensor("b0", (DH,), F32, kind="ExternalInput")
    b1 = nc.dram_tensor("b1", (DH,), F32, kind="ExternalInput")
    b2 = nc.dram_tensor("b2", (DOUT,), F32, kind="ExternalInput")
    out = nc.dram_tensor("out", (S, DOUT), F32, kind="ExternalOutput")

    K0 = DIN // 128   # 4  k-tiles, layer 0
    K1 = DH // 128    # 8  k-tiles, layers 1/2
    M0 = DH // 128    # 8  m-tiles (feature tiles of h0/h1)
    MT = SC // 128    # 4  seq m-tiles per chunk, layer 2
    NCH = S // SC     # 8  chunks

    Relu = mybir.ActivationFunctionType.Relu

    with tile.TileContext(nc) as tc:
        with (
            tc.tile_pool(name="weights", bufs=1) as wpool,
            tc.tile_pool(name="biases", bufs=1) as bpool,
            tc.tile_pool(name="acts", bufs=2) as apool,
            tc.tile_pool(name="qin", bufs=2) as qpool,
            tc.tile_pool(name="outp", bufs=4) as opool,
            tc.tile_pool(name="psum0", bufs=2, space="PSUM") as ppool0,
            tc.tile_pool(name="psum1", bufs=3, space="PSUM") as ppool1,
            tc.tile_pool(name="psum2", bufs=3, space="PSUM") as ppool2,
        ):
            # Pre-warm the PE clock gate (HAM) with dummy matmuls on garbage
            # data while the startup DMAs land (the memsets land ~8us in,
            # after the DVE preamble, which matches when the DMA rings go
            # live): the real matmul stream then starts at 2.4 GHz.
            g_lhs = apool.tile([128, 128], BF16, tag="warm_lhs")
            g_rhs = apool.tile([128, SC], BF16, tag="warm_rhs")
            nc.vector.memset(g_lhs, 0.0)
            nc.vector.memset(g_rhs, 0.0)
            warm_ps = ppool0.tile([128, SC], F32, tag="ps0")
            N_WARM = 12
            for i in range(N_WARM):
                nc.tensor.matmul(
                    warm_ps, lhsT=g_lhs, rhs=g_rhs,
                    start=(i == 0), stop=(i == N_WARM - 1),
                )

            # Startup-critical loads: layer-0 weights + the first two seq
            # chunks on the Sync engine's HWDGE ring; W1/W2 go out on the
            # Scalar engine's ring in parallel (one dynamic HWDGE ring per
            # issuing engine, ~150-265 GB/s each, live only after the ~8us
            # engine preamble). W1 is split across both rings so its last
            # tile lands before chunk-0 layer-1 needs it.
            w0_sb = [wpool.tile([128, DH], F32R, tag=f"w0_{k}", name=f"w0_{k}") for k in range(K0)]
            q0_sb = [qpool.tile([128, SC], F32R, tag=f"q_{k}", name=f"q0_{k}") for k in range(K0)]
            for k in range(K0):
                nc.sync.dma_start(out=w0_sb[k], in_=w0t[k * 128:(k + 1) * 128, :])
                nc.sync.dma_start(out=q0_sb[k], in_=qT[k * 128:(k + 1) * 128, 0:SC])
            b0_sb = bpool.tile([128, M0], F32, tag="b0")
            nc.gpsimd.dma_start(out=b0_sb, in_=b0[:].rearrange("(m p) -> p m", p=128))

            q1_sb = []
            for k in range(K0):
                t = qpool.tile([128, SC], F32R, tag=f"q_{k}", name=f"q1pre_{k}")
                nc.sync.dma_start(out=t, in_=qT[k * 128:(k + 1) * 128, SC:2 * SC])
                q1_sb.append(t)

            w1_sb = [wpool.tile([128, DH], F32R, tag=f"w1_{k}", name=f"w1_{k}") for k in range(K1)]
            for k in range(K1):
                eng = nc.sync if k % 2 == 0 else nc.scalar
                eng.dma_start(out=w1_sb[k], in_=w1t[k * 128:(k + 1) * 128, :])
            b1_sb = bpool.tile([128, M0], F32, tag="b1")
            nc.gpsimd.dma_start(out=b1_sb, in_=b1[:].rearrange("(m p) -> p m", p=128))

            w2_sb = [wpool.tile([128, DOUT], F32R, tag=f"w2_{k}", name=f"w2_{k}") for k in range(K1)]
            for k in range(K1):
                nc.scalar.dma_start(out=w2_sb[k], in_=w2t[k * 128:(k + 1) * 128, :])
            b2_sb = bpool.tile([128, DOUT], F32, tag="b2")
            b2_ap = b2[:]
            b2_bcast = bass.AP(
                tensor=b2_ap.tensor,
                offset=b2_ap.offset,
                ap=[[0, 128]] + [list(d) for d in b2_ap.ap],
            )
            nc.gpsimd.dma_start(out=b2_sb, in_=b2_bcast)

            def load_q(c):
                s0 = c * SC
                q_sb = []
                for k in range(K0):
                    t = qpool.tile([128, SC], F32R, tag=f"q_{k}", name=f"q{c}_{k}")
                    nc.sync.dma_start(
                        out=t, in_=qT[k * 128:(k + 1) * 128, s0:s0 + SC]
                    )
                    q_sb.append(t)
                return q_sb

            def layer0(c, q_sb):
                h0_sb = []
                for m in range(M0):
                    ps = ppool0.tile([128, SC], F32, tag="ps0", name=f"ps0_{c}_{m}")
                    for k in range(K0):
                        nc.tensor.matmul(
                            ps,
                            lhsT=w0_sb[k][:, m * 128:(m + 1) * 128],
                            rhs=q_sb[k],
                            start=(k == 0),
                            stop=(k == K0 - 1),
                        )
                    h = apool.tile([128, SC], F32R, tag=f"h0_{m}", name=f"h0_{c}_{m}")
                    nc.scalar.activation(h, ps, Relu, bias=b0_sb[:, m:m + 1])
                    h0_sb.append(h)
                return h0_sb

            def layers12(c, h0_sb):
                s0 = c * SC
                h1_sb = []
                for m in range(M0):
                    ps = ppool1.tile([128, SC], F32, tag="ps1", name=f"ps1_{c}_{m}")
                    for k in range(K1):
                        nc.tensor.matmul(
                            ps,
                            lhsT=w1_sb[k][:, m * 128:(m + 1) * 128],
                            rhs=h0_sb[k],
                            start=(k == 0),
                            stop=(k == K1 - 1),
                        )
                    h = apool.tile([128, SC], F32R, tag=f"h1_{m}", name=f"h1_{c}_{m}")
                    nc.scalar.activation(h, ps, Relu, bias=b1_sb[:, m:m + 1])
                    h1_sb.append(h)

                for mt in range(MT):
                    ps = ppool2.tile([128, DOUT], F32, tag="ps2", name=f"ps2_{c}_{mt}")
                    for k in range(K1):
                        nc.tensor.matmul(
                            ps,
                            lhsT=h1_sb[k][:, mt * 128:(mt + 1) * 128],
                            rhs=w2_sb[k],
                            start=(k == 0),
                            stop=(k == K1 - 1),
                        )
                    ot = opool.tile([128, DOUT], F32, tag="ot", name=f"ot_{c}_{mt}")
                    nc.vector.tensor_add(ot, ps, b2_sb)
                    eng = nc.scalar if mt % 2 == 0 else nc.sync
                    eng.dma_start(
                        out=out[s0 + mt * 128:s0 + (mt + 1) * 128, :], in_=ot
                    )

            # Software pipeline: emit L0 of chunk c+1 ahead of L1/L2 of
            # chunk c, so the matmul stream never depends on a DMA issued
            # less than a full chunk earlier.
            h0_cur = layer0(0, q0_sb)
            for c in range(NCH):
                h0_next = None
                if c + 1 < NCH:
                    q_sb = q1_sb if c + 1 == 1 else load_q(c + 1)
                    h0_next = layer0(c + 1, q_sb)
                layers12(c, h0_cur)
                h0_cur = h0_next
    nc.finalize()
    return nc


_NC = None


def _get_nc():
    global _NC
    if _NC is None:
        _NC = build_nc()
    return _NC


def make_in_maps(inputs):
    bf16 = ml_dtypes.bfloat16
    q, W0, b0, W1, b1, W2, b2 = (
        inputs["query"], inputs["W0"], inputs["b0"], inputs["W1"],
        inputs["b1"], inputs["W2"], inputs["b2"],
    )
    in_maps = []
    for b in range(B):
        in_maps.append({
            "qT": np.ascontiguousarray(np.asarray(q[b]).T, dtype=np.float32),
            "w0t": np.ascontiguousarray(np.asarray(W0[b]).T, dtype=np.float32),
            "w1t": np.ascontiguousarray(np.asarray(W1[b]).T, dtype=np.float32),
            "w2t": np.ascontiguousarray(np.asarray(W2[b]).T, dtype=np.float32),
            "b0": np.asarray(b0[b], dtype=np.float32),
            "b1": np.asarray(b1[b], dtype=np.float32),
            "b2": np.asarray(b2[b], dtype=np.float32),
        })
    return in_maps


def run(inputs, trace=False):
    nc = _get_nc()
    in_maps = make_in_maps(inputs)
    res = run_bass_kernel_spmd(nc, in_maps, core_ids=list(range(B)), trace=trace)
    out = np.stack([np.asarray(r["out"], dtype=np.float32) for r in res.results])
    return out, res


def kernel(**inputs) -> np.ndarray:
    out, _ = run(inputs, trace=False)
    return out



# revision 4
# speedup vs baseline: 1.0793x; 1.0793x over previous
"""Trainium2 Bass kernel for nn_LongTermMemoryMLP.

Per-batch-weight 3-layer MLP:
    h0 = relu(q @ W0^T + b0); h1 = relu(h0 @ W1^T + b1); out = h1 @ W2^T + b2
with q: [B,S,DIN], W0: [B,DH,DIN], W1: [B,DH,DH], W2: [B,DOUT,DH], B=8.

Sharding: data-parallel over batch — one batch sample (and its weight slabs)
per NeuronCore, 8 cores, no cross-core communication.

Device-side strategy (v2): everything feature-major. Activations live as
[feature, seq] tiles (feature on partitions), weights are pre-transposed on
the host, so every layer is stationary=weight-slice, moving=activation —
including layer 2, whose output lands transposed ([DOUT, S]) and is
un-transposed on the host. That makes every bias a per-partition scalar,
applied for free in the scalar-engine activation that drains PSUM (no
broadcast-bias DMA, vector engine freed up for stores).

All matmul operands are bf16 (tolerance is 2e-2; measured pipeline error
~4.3e-3): halves HBM traffic vs fp32r (startup-critical) and enables the
fast-weight-load path on LDWEIGHTS. PSUM accumulation stays fp32.

Startup is the other big lever (the fixed engine preamble ends ~6.5us, and
the PE must be streaming real matmuls as soon after as possible): the
layer-0 weights and first seq chunk are spread across all four DMA queues
(sync/scalar HWDGE, gpsimd/vector SWDGE) so the first k-groups' operands
land ~2.5us after the rings go live, with a short burst of dummy bf16
matmuls keeping the PE-HAM clock gate warm until they do.
"""

import numpy as np

import ml_dtypes

import concourse.bass as bass
import concourse.tile as tile
from concourse import bacc, mybir
from concourse.bass_utils import run_bass_kernel_spmd

B, S, DIN, DH, DOUT = 8, 4096, 512, 1024, 512
SC = 512  # seq chunk processed per pipeline iteration

BF16 = mybir.dt.bfloat16
F32 = mybir.dt.float32

K0 = DIN // 128   # 4  k-tiles, layer 0
K1 = DH // 128    # 8  k-tiles, layers 1/2
M0 = DH // 128    # 8  m-tiles (feature tiles of h0/h1)
M2 = DOUT // 128  # 4  m-tiles (feature tiles of outT)
NCH = S // SC     # 8  chunks

N_WARM = 12       # dummy bf16 matmuls bridging preamble-end -> first data


def build_nc():
    nc = bacc.Bacc("TRN2")
    qT = nc.dram_tensor("qT", (DIN, S), BF16, kind="ExternalInput")
    w0t = nc.dram_tensor("w0t", (DIN, DH), BF16, kind="ExternalInput")
    w1t = nc.dram_tensor("w1t", (DH, DH), BF16, kind="ExternalInput")
    w2t = nc.dram_tensor("w2t", (DH, DOUT), BF16, kind="ExternalInput")
    b0 = nc.dram_tensor("b0", (DH,), F32, kind="ExternalInput")
    b1 = nc.dram_tensor("b1", (DH,), F32, kind="ExternalInput")
    b2 = nc.dram_tensor("b2", (DOUT,), F32, kind="ExternalInput")
    outT = nc.dram_tensor("outT", (DOUT, S), F32, kind="ExternalOutput")

    Relu = mybir.ActivationFunctionType.Relu
    Ident = mybir.ActivationFunctionType.Identity

    with tile.TileContext(nc) as tc:
        with (
            tc.tile_pool(name="weights", bufs=1) as wpool,
            tc.tile_pool(name="biases", bufs=1) as bpool,
            tc.tile_pool(name="acts", bufs=2) as apool,
            tc.tile_pool(name="qin", bufs=3) as qpool,
            tc.tile_pool(name="outp", bufs=4) as opool,
            tc.tile_pool(name="psum0", bufs=2, space="PSUM") as ppool0,
            tc.tile_pool(name="psum1", bufs=3, space="PSUM") as ppool1,
            tc.tile_pool(name="psum2", bufs=3, space="PSUM") as ppool2,
        ):
            # Tiny warm tiles for the HAM warm-up matmuls.
            g_lhs = apool.tile([128, 128], BF16, tag="warm_lhs")
            g_rhs = apool.tile([128, 256], BF16, tag="warm_rhs")
            nc.vector.memset(g_lhs, 0.0)
            nc.vector.memset(g_rhs, 0.0)

            # ---- startup loads, spread across the three DMA queues ----
            # (sync/scalar HWDGE + gpsimd SWDGE). Biases go out first (tiny
            # scatter patterns, needed by the first relu); the first
            # k-group operands (w0[k], q0[k]) land round-robin so the real
            # matmul stream can start ~3us after the rings go live.
            w0_sb = [wpool.tile([128, DH], BF16, tag=f"w0_{k}", name=f"w0_{k}") for k in range(K0)]
            q0_sb = [qpool.tile([128, SC], BF16, tag=f"q_{k}", name=f"q0_{k}") for k in range(K0)]

            b0_sb = bpool.tile([128, M0], F32, tag="b0")
            nc.sync.dma_start(out=b0_sb, in_=b0[:].rearrange("(m p) -> p m", p=128))
            b1_sb = bpool.tile([128, M0], F32, tag="b1")
            nc.scalar.dma_start(out=b1_sb, in_=b1[:].rearrange("(m p) -> p m", p=128))

            nc.sync.dma_start(out=q0_sb[0], in_=qT[0:128, 0:SC])
            nc.scalar.dma_start(out=q0_sb[1], in_=qT[128:256, 0:SC])
            nc.gpsimd.dma_start(out=q0_sb[2], in_=qT[256:384, 0:SC])
            nc.gpsimd.dma_start(out=q0_sb[3], in_=qT[384:512, 0:SC])
            nc.sync.dma_start(out=w0_sb[0], in_=w0t[0:128, :])
            nc.scalar.dma_start(out=w0_sb[1], in_=w0t[128:256, :])
            nc.gpsimd.dma_start(out=w0_sb[2], in_=w0t[256:384, :])
            nc.gpsimd.dma_start(out=w0_sb[3], in_=w0t[384:512, :])

            # W1 split across both HWDGE rings right behind W0.
            w1_sb = [wpool.tile([128, DH], BF16, tag=f"w1_{k}", name=f"w1_{k}") for k in range(K1)]
            for k in range(K1):
                eng = nc.sync if k % 2 == 0 else nc.scalar
                eng.dma_start(out=w1_sb[k], in_=w1t[k * 128:(k + 1) * 128, :])

            # Second chunk of q behind w0 on gpsimd; later chunks stream on
            # gpsimd one chunk ahead of compute.
            q1_sb = []
            for k in range(K0):
                t = qpool.tile([128, SC], BF16, tag=f"q_{k}", name=f"q1pre_{k}")
                nc.gpsimd.dma_start(out=t, in_=qT[k * 128:(k + 1) * 128, SC:2 * SC])
                q1_sb.append(t)

            w2_sb = [wpool.tile([128, DOUT], BF16, tag=f"w2_{k}", name=f"w2_{k}") for k in range(K1)]
            for k in range(K1):
                eng = nc.sync if k % 2 == 0 else nc.scalar
                eng.dma_start(out=w2_sb[k], in_=w2t[k * 128:(k + 1) * 128, :])

            b2_sb = bpool.tile([128, M2], F32, tag="b2")
            nc.gpsimd.dma_start(out=b2_sb, in_=b2[:].rearrange("(m p) -> p m", p=128))

            # ---- HAM warm-up: keep PE busy from preamble-end until the
            # first real operands land (dummy matmuls, garbage data).
            warm_ps = ppool0.tile([128, SC], F32, tag="ps0")
            for i in range(N_WARM):
                nc.tensor.matmul(
                    warm_ps[:, 0:256], lhsT=g_lhs, rhs=g_rhs,
                    start=(i == 0), stop=(i == N_WARM - 1),
                )

            def load_q(c):
                s0 = c * SC
                q_sb = []
                for k in range(K0):
                    t = qpool.tile([128, SC], BF16, tag=f"q_{k}", name=f"q{c}_{k}")
                    nc.gpsimd.dma_start(
                        out=t, in_=qT[k * 128:(k + 1) * 128, s0:s0 + SC]
                    )
                    q_sb.append(t)
                return q_sb

            def layer0(c, q_sb):
                h0_sb = []
                for m in range(M0):
                    ps = ppool0.tile([128, SC], F32, tag="ps0", name=f"ps0_{c}_{m}")
                    for k in range(K0):
                        nc.tensor.matmul(
                            ps,
                            lhsT=w0_sb[k][:, m * 128:(m + 1) * 128],
                            rhs=q_sb[k],
                            start=(k == 0),
                            stop=(k == K0 - 1),
                        )
                    h = apool.tile([128, SC], BF16, tag=f"h0_{m}", name=f"h0_{c}_{m}")
                    nc.scalar.activation(h, ps, Relu, bias=b0_sb[:, m:m + 1])
                    h0_sb.append(h)
                return h0_sb

            def layers12(c, h0_sb):
                s0 = c * SC
                h1_sb = []
                for m in range(M0):
                    ps = ppool1.tile([128, SC], F32, tag="ps1", name=f"ps1_{c}_{m}")
                    for k in range(K1):
                        nc.tensor.matmul(
                            ps,
                            lhsT=w1_sb[k][:, m * 128:(m + 1) * 128],
                            rhs=h0_sb[k],
                            start=(k == 0),
                            stop=(k == K1 - 1),
                        )
                    h = apool.tile([128, SC], BF16, tag=f"h1_{m}", name=f"h1_{c}_{m}")
                    nc.scalar.activation(h, ps, Relu, bias=b1_sb[:, m:m + 1])
                    h1_sb.append(h)

                for md in range(M2):
                    ps = ppool2.tile([128, SC], F32, tag="ps2", name=f"ps2_{c}_{md}")
                    for k in range(K1):
                        nc.tensor.matmul(
                            ps,
                            lhsT=w2_sb[k][:, md * 128:(md + 1) * 128],
                            rhs=h1_sb[k],
                            start=(k == 0),
                            stop=(k == K1 - 1),
                        )
                    ot = opool.tile([128, SC], F32, tag="ot", name=f"ot_{c}_{md}")
                    nc.scalar.activation(ot, ps, Ident, bias=b2_sb[:, md:md + 1])
                    nc.sync.dma_start(
                        out=outT[md * 128:(md + 1) * 128, s0:s0 + SC], in_=ot
                    )

            # Software pipeline: emit L0 of chunk c+1 ahead of L1/L2 of
            # chunk c, so the matmul stream never depends on a DMA issued
            # less than a full chunk earlier.
            h0_cur = layer0(0, q0_sb)
            for c in range(NCH):
                h0_next = None
                if c + 1 < NCH:
                    q_sb = q1_sb if c + 1 == 1 else load_q(c + 1)
                    h0_next = layer0(c + 1, q_sb)
                layers12(c, h0_cur)
                h0_cur = h0_next
    nc.finalize()
    return nc


_NC = None


def _get_nc():
    global _NC
    if _NC is None:
        _NC = build_nc()
    return _NC


def make_in_maps(inputs):
    bf16 = ml_dtypes.bfloat16
    q, W0, b0, W1, b1, W2, b2 = (
        inputs["query"], inputs["W0"], inputs["b0"], inputs["W1"],
        inputs["b1"], inputs["W2"], inputs["b2"],
    )
    in_maps = []
    for b in range(B):
        in_maps.append({
            "qT": np.ascontiguousarray(np.asarray(q[b]).T).astype(bf16),
            "w0t": np.ascontiguousarray(np.asarray(W0[b]).T).astype(bf16),
            "w1t": np.ascontiguousarray(np.asarray(W1[b]).T).astype(bf16),
            "w2t": np.ascontiguousarray(np.asarray(W2[b]).T).astype(bf16),
            "b0": np.asarray(b0[b], dtype=np.float32),
            "b1": np.asarray(b1[b], dtype=np.float32),
            "b2": np.asarray(b2[b], dtype=np.float32),
        })
    return in_maps


def run(inputs, trace=False):
    nc = _get_nc()
    in_maps = make_in_maps(inputs)
    res = run_bass_kernel_spmd(nc, in_maps, core_ids=list(range(B)), trace=trace)
    out = np.stack(
        [np.asarray(r["outT"], dtype=np.float32).T for r in res.results]
    )
    return np.ascontiguousarray(out), res


def kernel(**inputs) -> np.ndarray:
    out, _ = run(inputs, trace=False)
    return out
